# revision 40
# baseline (speedup 1.0000x reference)
"""Trainium2 Bass kernel for nn_BertReshapeAttention (sparse slot attention).

Strategy: data-parallel over the B=8 dialogue axis — one dialogue per
NeuronCore; the 768x768 projection weights are replicated.

Per core (dialogue b), with S=30 slots x L=16 tokens = 480 query positions:
  QT/KT = W^T @ X^T  (transposed layout, head dim on partitions)
  V     = X @ Wv     (natural layout)
  Per head: S1T = CK_h @ Q_h^T  (cache keys on partitions, queries on free)
            P1  = exp(S1T*scale);  savU = [CV_h | 1]^T @ P1  (unnormalized
            seq_att_value with the softmax denominator D1 as an extra row,
            via a ones-column appended to CV)
  sav_norm = savU / D1;  EKT = Wek^T @ sav_norm^T;  EV = sav_norm @ Wev
  Per head: S2T = EK_h @ Q_h^T (slot scores), S3T = block-diag local scores
            ctxT = I65 @ savU_h + [EV_h|1]^T @ P2 + [V_h|1]^T @ P3
            (the cache-part numerator of the big softmax is exactly savU,
            and its denominator contribution is exactly D1 — both folded in
            via a single 65x65-identity matmul that opens the PSUM group)
  out_h = ctxT_h / D2, written transposed bf16; the host casts+transposes.

Layout/scheduling choices (v2, from the perfetto trace of the f32r v1):
  * all matmul operands bf16 (1 PE cycle/row vs 2 for f32r; FWL weight
    loads) — fp32 PSUM accumulation keeps rel-err ~1e-3 vs the 2e-2 gate
  * per-head score matmuls have K=64: even heads live at partitions 0..63,
    odd heads at 64..127, and the pair's matmuls are issue-interleaved so
    they run CONCURRENTLY in different PE row-groups (~2x on S1/S2/S3)
  * phase structure Q-proj -> cache attention -> EK/EV proj -> final
    softmax+context, with the K/V projections issued as PE filler inside
    the ACT-bound cache-attention phase and EV inside the final phase
  * PSUM budget (8 banks): phase A uses one 6-bank tile (k-outer Q GEMM,
    single batched cast); phases B-D use scores 2x[128,2,512] (4 banks) +
    sav/ctx 2x[65,480] (2) + aux 2x[128,512] (2)
"""
import os
import numpy as np

import concourse.bass as bass
import concourse.mybir as mybir
import concourse.tile as tile
from concourse import bacc
from concourse.bass_utils import run_bass_kernel_spmd

dt = mybir.dt
AF = mybir.ActivationFunctionType

H, HD = 12, 64            # heads, head dim
S, B, L, D, SEQ = 30, 8, 16, 768, 512
NQ = S * L                # 480 queries per core
CH = [128, 128, 128, 96]  # query/slot-key chunk sizes (slot-aligned)
CO = [0, 128, 256, 384]
KC = D // 128             # 6 contraction chunks
SCALE = 1.0 / 8.0         # 1/sqrt(HD)
F32 = dt.float32
BF16 = dt.bfloat16
MMDT = dt.float32r if os.environ.get("BASS_MM_DT") == "f32r" else BF16

N_CORES = 8


def build_bass():
    nc = bacc.Bacc("TRN2")

    # all inputs pre-arranged host-side into the exact SBUF layout so each
    # is ONE contiguous-descriptor DMA
    xt = nc.dram_tensor("xt", (128, KC, NQ), MMDT, kind="ExternalInput")
    # paired cache keys: [0:64,j]=CK_{2j}^T, [64:128,j]=CK_{2j+1}^T
    ckp = nc.dram_tensor("ckp", (128, KC, SEQ), MMDT, kind="ExternalInput")
    # cache_value chunks with a ones-column per head: [64 values | 1]
    cva = nc.dram_tensor("cva", (128, H, 4, 65), MMDT, kind="ExternalInput")
    wq = nc.dram_tensor("wq", (128, KC, D), MMDT, kind="ExternalInput")
    wk = nc.dram_tensor("wk", (128, KC, D), MMDT, kind="ExternalInput")
    wv = nc.dram_tensor("wv", (128, KC, D), MMDT, kind="ExternalInput")
    wek = nc.dram_tensor("wek", (128, KC, D), MMDT, kind="ExternalInput")
    wev = nc.dram_tensor("wev", (128, KC, D), MMDT, kind="ExternalInput")
    outt = nc.dram_tensor("outt", (D, NQ), F32, kind="ExternalOutput")

    with tile.TileContext(nc) as tc, nc.allow_low_precision(
            reason="bf16 matmuls with fp32 PSUM; rel tolerance is 2e-2"):
        _build_body(tc, nc, xt, ckp, cva, wq, wk, wv, wek, wev, outt)
    nc.compile()
    return nc


def _build_body(tc, nc, xt, ckp, cva, wq, wk, wv, wek, wev, outt):
    with (
        tc.tile_pool(name="persist", bufs=1) as pers,
        tc.tile_pool(name="probs", bufs=4) as ppool,
        tc.tile_pool(name="small", bufs=2) as spool,
        tc.tile_pool(name="outp", bufs=2) as outpool,
    ):
        # ---- persistent SBUF tiles ----
        xts = pers.tile([128, KC, NQ], MMDT)      # X^T chunks
        qts = pers.tile([128, KC, NQ], MMDT)      # Q^T
        kts = pers.tile([128, KC, NQ], MMDT)      # K^T
        vs = pers.tile([128, 4, H, 65], MMDT)     # V natural, ones-augmented
        evs = pers.tile([128, 4, H, 65], MMDT)    # EV natural, ones-augmented
        savus = pers.tile([65, H, NQ], MMDT)      # unnorm sav^T + D1 row 64
        savn = pers.tile([128, KC, NQ], MMDT)     # normalized sav^T chunks
        ekts = pers.tile([128, KC, NQ], MMDT)     # EK^T
        cks = pers.tile([128, KC, SEQ], MMDT)     # paired cache keys^T
        cvas = pers.tile([128, H, 4, 65], MMDT)   # ones-augmented cache vals
        wqs = pers.tile([128, KC, D], MMDT)       # whole weight matrices
        wks = pers.tile([128, KC, D], MMDT)
        wvs = pers.tile([128, KC, D], MMDT)
        weks = pers.tile([128, KC, D], MMDT)
        wevs = pers.tile([128, KC, D], MMDT)
        ones_bc = pers.tile([65, 64], MMDT)       # row 64 used as K=1 lhsT
        i65 = pers.tile([65, 65], MMDT)           # identity: savU+D1 fold-in
        bmask = pers.tile([128, 4, 128], MMDT)    # 16x16 block-diag 0/1 mask

        # minimal staging for the PE warm-up (keep the DVE progress counter
        # short: warm-up waits only on these first few DVE instructions)
        wtile = pers.tile([65, NQ], MMDT)
        nc.vector.memset(wtile, 0.0)
        ones_f32 = pers.tile([65, 64], F32)
        nc.vector.memset(ones_f32, 1.0)
        nc.vector.tensor_copy(ones_bc, ones_f32)

        # ---- input DMAs: one FIFO ring (Sync), strictly in consumption
        # order so early phases never compete with late weights for HBM
        # bandwidth; wek/wev are issued later from the Scalar stream.
        nc.sync.dma_start(out=xts[:, 0:3, :], in_=xt[:, 0:3, :])
        nc.sync.dma_start(out=wqs[:, 0:3, :], in_=wq[:, 0:3, :])
        nc.sync.dma_start(out=xts[:, 3:6, :], in_=xt[:, 3:6, :])
        nc.sync.dma_start(out=wqs[:, 3:6, :], in_=wq[:, 3:6, :])
        nc.sync.dma_start(out=cks, in_=ckp[:, :, :])
        nc.sync.dma_start(out=cvas, in_=cva[:, :, :, :])
        nc.sync.dma_start(out=wks, in_=wk[:, :, :])
        nc.sync.dma_start(out=wvs, in_=wv[:, :, :])

        with (
            tc.tile_pool(name="scorep", bufs=2, space="PSUM") as scorep,
            tc.tile_pool(name="savp", bufs=2, space="PSUM") as savp,
            tc.tile_pool(name="auxp", bufs=2, space="PSUM") as auxp,
        ):
            def gemm_mstep(psum, lhsT_of_k, rhs_of_k, dst_copy):
                for k in range(KC):
                    nc.tensor.matmul(psum, lhsT_of_k(k), rhs_of_k(k),
                                     start=(k == 0), stop=(k == KC - 1))
                dst_copy(psum)

            # HAM warm-up: junk matmuls with no DMA deps run while the
            # input DMAs land, so the real GEMM starts closer to 2.4 GHz
            wps = auxp.tile([128, 512], F32, tag="aux", name="warm")
            for w in range(17):
                nc.tensor.matmul(wps[0:64, :NQ], ones_bc, wtile,
                                 start=True, stop=True)

            # ============ Phase A: Q projection (m-outer) ============
            for m in range(KC):
                ps = auxp.tile([128, 512], F32, tag="aux", name="qps")
                gemm_mstep(
                    ps[:, :NQ],
                    lambda k, m=m: wqs[:, k, m * 128:(m + 1) * 128],
                    lambda k: xts[:, k, :],
                    lambda p, m=m: nc.vector.tensor_copy(qts[:, m, :], p))

            # late constants (first used in phases B tail / C / D)
            onescol = pers.tile([128, 4, H, 1], F32)
            nc.vector.memset(onescol, 1.0)
            nc.vector.tensor_copy(vs[:, :, :, 64:65], onescol)
            nc.vector.tensor_copy(evs[:, :, :, 64:65], onescol)

            i65_f = pers.tile([65, 65], F32)
            nc.vector.memset(i65_f, 1.0)
            nc.gpsimd.affine_select(
                out=i65_f, in_=i65_f, compare_op=mybir.AluOpType.is_ge,
                fill=0.0, base=0, channel_multiplier=1, pattern=[[-1, 65]])
            nc.gpsimd.affine_select(
                out=i65_f, in_=i65_f, compare_op=mybir.AluOpType.is_ge,
                fill=0.0, base=0, channel_multiplier=-1, pattern=[[1, 65]])
            nc.vector.tensor_copy(i65, i65_f)

            # block-diag mask: 1.0 where key//16 == query//16 within a chunk
            bmask_f = pers.tile([128, 4, 128], F32)
            nc.vector.memset(bmask_f, 1.0)
            nc.gpsimd.affine_select(
                out=bmask_f, in_=bmask_f, compare_op=mybir.AluOpType.is_ge,
                fill=0.0, base=0, channel_multiplier=1,
                pattern=[[0, 4], [-16, 8], [0, 16]])
            nc.gpsimd.affine_select(
                out=bmask_f, in_=bmask_f, compare_op=mybir.AluOpType.is_ge,
                fill=0.0, base=15, channel_multiplier=-1,
                pattern=[[0, 4], [16, 8], [0, 16]])
            nc.vector.tensor_copy(bmask, bmask_f)

            # V[qc-chunk, hh] = sum_k XT[k][:,qc]^T @ Wv[k][:, hh*384:...]
            vsteps = [(qc, hh) for qc in range(4) for hh in range(2)]

            def v_step(qc, hh):
                cw = CH[qc]
                ps = auxp.tile([128, 512], F32, tag="aux", name="vps")
                gemm_mstep(
                    ps[:cw, :384],
                    lambda k: xts[:, k, CO[qc]:CO[qc] + cw],
                    lambda k: wvs[:, k, hh * 384:(hh + 1) * 384],
                    lambda p: nc.vector.tensor_copy(
                        vs[:cw, qc, hh * 6:(hh + 1) * 6, 0:64],
                        p.rearrange("p (h hd) -> p h hd", hd=64)))

            # ==== Phase B: cache attention head pairs + K/V GEMM filler ====
            for j in range(KC):
                he, ho = 2 * j, 2 * j + 1
                qe = qts[0:64, j, :]
                qo = qts[64:128, j, :]
                p1e = ppool.tile([128, 4, NQ], MMDT, tag="p1", name="p1e")
                p1o = ppool.tile([128, 4, NQ], MMDT, tag="p1", name="p1o")
                for half in range(2):
                    se = scorep.tile([128, 2, 512], F32, tag="score",
                                     name="s1e")
                    so = scorep.tile([128, 2, 512], F32, tag="score",
                                     name="s1o")
                    for i in range(2):
                        c = 2 * half + i
                        nc.tensor.matmul(
                            se[:, i, :NQ],
                            cks[0:64, j, c * 128:(c + 1) * 128], qe,
                            start=True, stop=True)
                        nc.tensor.matmul(
                            so[:, i, :NQ],
                            cks[64:128, j, c * 128:(c + 1) * 128], qo,
                            start=True, stop=True)
                    nc.scalar.activation(p1e[:, 2 * half:2 * half + 2, :],
                                         se[:, :, :NQ], AF.Exp, scale=SCALE)
                    nc.scalar.activation(p1o[:, 2 * half:2 * half + 2, :],
                                         so[:, :, :NQ], AF.Exp, scale=SCALE)

                # K-projection m-chunk as PE filler
                ps = auxp.tile([128, 512], F32, tag="aux", name="kps")
                gemm_mstep(
                    ps[:, :NQ],
                    lambda k, j=j: wks[:, k, j * 128:(j + 1) * 128],
                    lambda k: xts[:, k, :],
                    lambda p, j=j: nc.vector.tensor_copy(kts[:, j, :], p))

                # savU + D1 row at partition 64 (per head, K=128 full array)
                for h, p1 in ((he, p1e), (ho, p1o)):
                    savps = savp.tile([65, NQ], F32, tag="sav", name="savps")
                    for c in range(4):
                        nc.tensor.matmul(savps, cvas[:, h, c, :], p1[:, c, :],
                                         start=(c == 0), stop=(c == 3))
                    nc.vector.tensor_copy(savus[:, h, :], savps)

                # odd head's sav values land at partitions 64..127 of savn
                nc.sync.dma_start(out=savn[64:128, j, :],
                                  in_=savus[0:64, ho, :])

                # 1/D1 broadcast over the pair's partitions via K=1 matmuls
                bcp = auxp.tile([128, 512], F32, tag="aux", name="bc1")
                nc.tensor.matmul(bcp[0:64, :NQ], ones_bc[64:65, :],
                                 savus[64:65, he, :])
                nc.tensor.matmul(bcp[64:128, :NQ], ones_bc[64:65, :],
                                 savus[64:65, ho, :])
                rbc = spool.tile([128, NQ], F32, tag="rbc", name="rbc")
                nc.vector.reciprocal_approx_fast(out=rbc, in_=bcp[:, :NQ])
                rbcb = spool.tile([128, NQ], MMDT, tag="rbcb", name="rbcb")
                nc.vector.tensor_copy(rbcb, rbc)
                nc.vector.tensor_mul(savn[0:64, j, :], savus[0:64, he, :],
                                     rbcb[0:64, :])
                nc.vector.tensor_mul(savn[64:128, j, :], savn[64:128, j, :],
                                     rbcb[64:128, :])

                # late-phase weights: issued from the Scalar stream once the
                # startup-critical transfers are done (FIFO per ring)
                if j == 1:
                    nc.scalar.dma_start(out=weks, in_=wek[:, :, :])
                if j == 2:
                    nc.scalar.dma_start(out=wevs, in_=wev[:, :, :])

                # V-projection filler: ~1.3 m-steps per pair
                if j < 4:
                    v_step(*vsteps[j])
                else:
                    v_step(*vsteps[2 * j - 4])
                    v_step(*vsteps[2 * j - 3])

            # ============ Phase C: EK / EV projections ============
            # ACT is idle here, so PSUM evacuation goes on nc.scalar
            for m in range(KC):
                ps = auxp.tile([128, 512], F32, tag="aux", name="ekps")
                gemm_mstep(
                    ps[:, :NQ],
                    lambda k, m=m: weks[:, k, m * 128:(m + 1) * 128],
                    lambda k: savn[:, k, :],
                    lambda p, m=m: nc.scalar.copy(ekts[:, m, :], p))

            def ev_step(qc, hh, eng):
                cw = CH[qc]
                ps = auxp.tile([128, 512], F32, tag="aux", name="evps")
                gemm_mstep(
                    ps[:cw, :384],
                    lambda k: savn[:, k, CO[qc]:CO[qc] + cw],
                    lambda k: wevs[:, k, hh * 384:(hh + 1) * 384],
                    lambda p: eng.tensor_copy(
                        evs[:cw, qc, hh * 6:(hh + 1) * 6, 0:64],
                        p.rearrange("p (h hd) -> p h hd", hd=64))
                    if eng is nc.vector else
                    nc.scalar.copy(
                        evs[:cw, qc, hh * 6:(hh + 1) * 6, 0:64],
                        p.rearrange("p (h hd) -> p h hd", hd=64)))

            # heads 0..5 (hh=0) are consumed from phase-D pair 0 on: all
            # four of their EV chunks must be issued BEFORE the first ctx
            ev_step(0, 0, nc.scalar)
            ev_step(1, 0, nc.scalar)
            ev_step(2, 0, nc.scalar)
            ev_step(3, 0, nc.scalar)

            # ============ Phase D: full softmax + context ============
            outg = None
            for j in range(KC):
                he, ho = 2 * j, 2 * j + 1
                qe = qts[0:64, j, :]
                qo = qts[64:128, j, :]
                p2e = ppool.tile([128, 4, NQ], MMDT, tag="p1", name="p2e")
                p2o = ppool.tile([128, 4, NQ], MMDT, tag="p1", name="p2o")
                for half in range(2):
                    se = scorep.tile([128, 2, 512], F32, tag="score",
                                     name="s2e")
                    so = scorep.tile([128, 2, 512], F32, tag="score",
                                     name="s2o")
                    for i in range(2):
                        c = 2 * half + i
                        cw = CH[c]
                        nc.tensor.matmul(
                            se[:cw, i, :NQ],
                            ekts[0:64, j, CO[c]:CO[c] + cw], qe,
                            start=True, stop=True)
                        nc.tensor.matmul(
                            so[:cw, i, :NQ],
                            ekts[64:128, j, CO[c]:CO[c] + cw], qo,
                            start=True, stop=True)
                    nc.scalar.activation(p2e[:, 2 * half:2 * half + 2, :],
                                         se[:, :, :NQ], AF.Exp, scale=SCALE)
                    nc.scalar.activation(p2o[:, 2 * half:2 * half + 2, :],
                                         so[:, :, :NQ], AF.Exp, scale=SCALE)

                # local block-diagonal scores, pair-interleaved (K=64)
                s3e = auxp.tile([128, 512], F32, tag="aux", name="s3e")
                s3o = auxp.tile([128, 512], F32, tag="aux", name="s3o")
                for c in range(4):
                    cw = CH[c]
                    nc.tensor.matmul(
                        s3e[:cw, c * 128:c * 128 + cw],
                        kts[0:64, j, CO[c]:CO[c] + cw],
                        qe[:, CO[c]:CO[c] + cw],
                        start=True, stop=True)
                    nc.tensor.matmul(
                        s3o[:cw, c * 128:c * 128 + cw],
                        kts[64:128, j, CO[c]:CO[c] + cw],
                        qo[:, CO[c]:CO[c] + cw],
                        start=True, stop=True)
                p3e = ppool.tile([128, 4, 128], MMDT, tag="p3", name="p3e",
                                 bufs=3)
                p3o = ppool.tile([128, 4, 128], MMDT, tag="p3", name="p3o",
                                 bufs=3)
                nc.scalar.activation(
                    p3e, s3e.rearrange("p (c n) -> p c n", n=128),
                    AF.Exp, scale=SCALE)
                nc.scalar.activation(
                    p3o, s3o.rearrange("p (c n) -> p c n", n=128),
                    AF.Exp, scale=SCALE)
                nc.vector.tensor_mul(p3e, p3e, bmask)
                nc.vector.tensor_mul(p3o, p3o, bmask)

                # EV-projection filler: hh=1 chunks, all issued before the
                # first hh=1 consumer (pair j=3, heads 6,7)
                if j < 2:
                    ev_step(2 * j, 1, nc.vector)
                    ev_step(2 * j + 1, 1, nc.vector)

                for h, p2, p3 in ((he, p2e, p3e), (ho, p2o, p3o)):
                    ctxps = savp.tile([65, NQ], F32, tag="sav", name="ctxps")
                    # savU + D1 fold-in via identity matmul opens the group
                    nc.tensor.matmul(ctxps, i65, savus[:, h, :],
                                     start=True, stop=False)
                    for c in range(4):
                        nc.tensor.matmul(ctxps, evs[:CH[c], c, h, :],
                                         p2[:CH[c], c, :],
                                         start=False, stop=False,
                                         skip_group_check=True)
                    for c in range(4):
                        cw = CH[c]
                        nc.tensor.matmul(
                            ctxps[:, CO[c]:CO[c] + cw],
                            vs[:cw, c, h, :], p3[:cw, c, :cw],
                            start=False, stop=(c == 3), skip_group_check=True)
                    # D2 lives in ctxps row 64; stage it in SBUF for the
                    # K=1 broadcast matmul (PE reads SBUF only)
                    d2row = spool.tile([65, NQ], MMDT, tag="cts",
                                       name="d2row", bufs=3)
                    nc.vector.tensor_copy(d2row[64:65, :], ctxps[64:65, :])
                    bcp2 = auxp.tile([64, 512], F32, tag="aux", name="bc2")
                    nc.tensor.matmul(bcp2[:, :NQ], ones_bc[64:65, :],
                                     d2row[64:65, :])
                    rbc2 = spool.tile([64, NQ], F32, tag="rbc", name="rbc2")
                    nc.vector.reciprocal_approx_fast(out=rbc2,
                                                     in_=bcp2[:, :NQ])
                    outg = outpool.tile([64, NQ], F32, tag="outg",
                                        name="outg", bufs=3)
                    nc.vector.tensor_mul(outg, ctxps[0:64, :], rbc2)
                    nc.sync.dma_start(out=outt[h * 64:(h + 1) * 64, :],
                                      in_=outg)


_BUILT = None


def _get_built():
    global _BUILT
    if _BUILT is None:
        _BUILT = build_bass()
    return _BUILT


last_exec_time_ns = None


def _np_mmdt():
    return dt.np(MMDT)


def make_cva(cv_b):
    """(12, 512, 64) cache values -> ones-augmented chunk layout."""
    cva = np.ones((128, H, 4, 65), np.float32)
    cva[:, :, :, 0:64] = cv_b.reshape(H, 4, 128, HD).transpose(2, 0, 1, 3)
    return cva.astype(_np_mmdt())


def make_ckp(ck_b):
    """(12, 512, 64) cache keys -> pair-packed transposed layout."""
    ckt = ck_b.transpose(0, 2, 1)                  # (H, HD, SEQ)
    ckp = np.empty((128, KC, SEQ), np.float32)
    for j in range(KC):
        ckp[0:64, j] = ckt[2 * j]
        ckp[64:128, j] = ckt[2 * j + 1]
    return ckp.astype(_np_mmdt())


def kernel(**inputs):
    global last_exec_time_ns
    hs = np.ascontiguousarray(np.asarray(inputs['hidden_states'],
                                         dtype=np.float32))
    ck = np.asarray(inputs['cache_key'], dtype=np.float32)
    cv = np.asarray(inputs['cache_value'], dtype=np.float32)
    ws = {k: np.ascontiguousarray(np.asarray(inputs[k], dtype=np.float32))
          for k in ('Wq', 'Wk', 'Wv', 'Wek', 'Wev')}

    for name in ('attention_mask', 'slot_unified_mask', 'bq', 'bk', 'bv',
                 'bek', 'bev'):
        if name in inputs and np.abs(np.asarray(inputs[name])).max() != 0:
            print(f"WARNING: kernel assumes {name} == 0 but it is not; "
                  f"results will be wrong")

    nc = _get_built()

    def sb_layout(a, inner):
        # (D, inner) -> contiguous (128, KC, inner) SBUF image
        return np.ascontiguousarray(
            a.reshape(KC, 128, inner).transpose(1, 0, 2)).astype(_np_mmdt())

    wsc = {k: sb_layout(w, D) for k, w in ws.items()}

    hs_r = hs.reshape(S, B, L, D)
    in_maps = []
    for b in range(N_CORES):
        in_maps.append({
            'xt': sb_layout(hs_r[:, b].reshape(NQ, D).T, NQ),
            'ckp': make_ckp(ck[b]),
            'cva': make_cva(cv[b]),
            'wq': wsc['Wq'], 'wk': wsc['Wk'], 'wv': wsc['Wv'],
            'wek': wsc['Wek'], 'wev': wsc['Wev'],
        })

    res = run_bass_kernel_spmd(
        nc, in_maps, core_ids=list(range(N_CORES)),
        trace=bool(os.environ.get("BASS_TRACE")),
        tmpdir=os.environ.get("BASS_TMPDIR"))
    last_exec_time_ns = res.exec_time_ns

    out = np.zeros((S, B, L, D), np.float32)
    for b in range(N_CORES):
        out[:, b] = res.results[b]['outt'].astype(np.float32).T.reshape(
            S, L, D)
    return out.reshape(S * B, L, D)


# revision 44
# speedup vs baseline: 1.0039x; 1.0039x over previous
"""Trainium2 Bass kernel for nn_BertReshapeAttention (sparse slot attention).

Strategy: data-parallel over the B=8 dialogue axis — one dialogue per
NeuronCore; the 768x768 projection weights are replicated.

Per core (dialogue b), with S=30 slots x L=16 tokens = 480 query positions:
  QT/KT = W^T @ X^T  (transposed layout, head dim on partitions)
  V     = X @ Wv     (natural layout)
  Per head: S1T = CK_h @ Q_h^T  (cache keys on partitions, queries on free)
            P1  = exp(S1T*scale);  savU = [CV_h | 1]^T @ P1  (unnormalized
            seq_att_value with the softmax denominator D1 as an extra row,
            via a ones-column appended to CV)
  sav_norm = savU / D1;  EKT = Wek^T @ sav_norm^T;  EV = sav_norm @ Wev
  Per head: S2T = EK_h @ Q_h^T (slot scores), S3T = block-diag local scores
            ctxT = I65 @ savU_h + [EV_h|1]^T @ P2 + [V_h|1]^T @ P3
            (the cache-part numerator of the big softmax is exactly savU,
            and its denominator contribution is exactly D1 — both folded in
            via a single 65x65-identity matmul that opens the PSUM group)
  out_h = ctxT_h / D2, written transposed bf16; the host casts+transposes.

Layout/scheduling choices (v2, from the perfetto trace of the f32r v1):
  * all matmul operands bf16 (1 PE cycle/row vs 2 for f32r; FWL weight
    loads) — fp32 PSUM accumulation keeps rel-err ~1e-3 vs the 2e-2 gate
  * per-head score matmuls have K=64: even heads live at partitions 0..63,
    odd heads at 64..127, and the pair's matmuls are issue-interleaved so
    they run CONCURRENTLY in different PE row-groups (~2x on S1/S2/S3)
  * phase structure Q-proj -> cache attention -> EK/EV proj -> final
    softmax+context, with the K/V projections issued as PE filler inside
    the ACT-bound cache-attention phase and EV inside the final phase
  * PSUM budget (8 banks): phase A uses one 6-bank tile (k-outer Q GEMM,
    single batched cast); phases B-D use scores 2x[128,2,512] (4 banks) +
    sav/ctx 2x[65,480] (2) + aux 2x[128,512] (2)
"""
import os
import numpy as np

import concourse.bass as bass
import concourse.mybir as mybir
import concourse.tile as tile
from concourse import bacc
from concourse.bass_utils import run_bass_kernel_spmd

dt = mybir.dt
AF = mybir.ActivationFunctionType

H, HD = 12, 64            # heads, head dim
S, B, L, D, SEQ = 30, 8, 16, 768, 512
NQ = S * L                # 480 queries per core
CH = [128, 128, 128, 96]  # query/slot-key chunk sizes (slot-aligned)
CO = [0, 128, 256, 384]
KC = D // 128             # 6 contraction chunks
SCALE = 1.0 / 8.0         # 1/sqrt(HD)
F32 = dt.float32
BF16 = dt.bfloat16
MMDT = dt.float32r if os.environ.get("BASS_MM_DT") == "f32r" else BF16

N_CORES = 8


def build_bass():
    nc = bacc.Bacc("TRN2")

    # all inputs pre-arranged host-side into the exact SBUF layout so each
    # is ONE contiguous-descriptor DMA
    xt = nc.dram_tensor("xt", (128, KC, NQ), MMDT, kind="ExternalInput")
    # paired cache keys: [0:64,j]=CK_{2j}^T, [64:128,j]=CK_{2j+1}^T
    ckp = nc.dram_tensor("ckp", (128, KC, SEQ), MMDT, kind="ExternalInput")
    # cache_value chunks with a ones-column per head: [64 values | 1]
    cva = nc.dram_tensor("cva", (128, H, 4, 65), MMDT, kind="ExternalInput")
    wq = nc.dram_tensor("wq", (128, KC, D), MMDT, kind="ExternalInput")
    wk = nc.dram_tensor("wk", (128, KC, D), MMDT, kind="ExternalInput")
    wv = nc.dram_tensor("wv", (128, KC, D), MMDT, kind="ExternalInput")
    wek = nc.dram_tensor("wek", (128, KC, D), MMDT, kind="ExternalInput")
    wev = nc.dram_tensor("wev", (128, KC, D), MMDT, kind="ExternalInput")
    outt = nc.dram_tensor("outt", (D, NQ), F32, kind="ExternalOutput")

    with tile.TileContext(nc) as tc, nc.allow_low_precision(
            reason="bf16 matmuls with fp32 PSUM; rel tolerance is 2e-2"):
        _build_body(tc, nc, xt, ckp, cva, wq, wk, wv, wek, wev, outt)
    nc.compile()
    return nc


def _build_body(tc, nc, xt, ckp, cva, wq, wk, wv, wek, wev, outt):
    with (
        tc.tile_pool(name="persist", bufs=1) as pers,
        tc.tile_pool(name="probs", bufs=4) as ppool,
        tc.tile_pool(name="small", bufs=2) as spool,
        tc.tile_pool(name="outp", bufs=2) as outpool,
    ):
        # ---- persistent SBUF tiles ----
        xts = pers.tile([128, KC, NQ], MMDT)      # X^T chunks
        qts = pers.tile([128, KC, NQ], MMDT)      # Q^T
        kts = pers.tile([128, KC, NQ], MMDT)      # K^T
        vs = pers.tile([128, 4, H, 65], MMDT)     # V natural, ones-augmented
        evs = pers.tile([128, 4, H, 65], MMDT)    # EV natural, ones-augmented
        savus = pers.tile([65, H, NQ], MMDT)      # unnorm sav^T + D1 row 64
        savn = pers.tile([128, KC, NQ], MMDT)     # normalized sav^T chunks
        ekts = pers.tile([128, KC, NQ], MMDT)     # EK^T
        cks = pers.tile([128, KC, SEQ], MMDT)     # paired cache keys^T
        cvas = pers.tile([128, H, 4, 65], MMDT)   # ones-augmented cache vals
        wqs = pers.tile([128, KC, D], MMDT)       # whole weight matrices
        wks = pers.tile([128, KC, D], MMDT)
        wvs = pers.tile([128, KC, D], MMDT)
        weks = pers.tile([128, KC, D], MMDT)
        wevs = pers.tile([128, KC, D], MMDT)
        ones_bc = pers.tile([65, 64], MMDT)       # row 64 used as K=1 lhsT
        i65 = pers.tile([65, 65], MMDT)           # identity: savU+D1 fold-in
        bmask = pers.tile([128, 4, 128], MMDT)    # 16x16 block-diag 0/1 mask

        # minimal staging for the PE warm-up (keep the DVE progress counter
        # short: warm-up waits only on these first few DVE instructions)
        wtile = pers.tile([65, NQ], MMDT)
        nc.vector.memset(wtile, 0.0)
        ones_f32 = pers.tile([65, 64], F32)
        nc.vector.memset(ones_f32, 1.0)
        nc.vector.tensor_copy(ones_bc, ones_f32)

        # ---- input DMAs: one FIFO ring (Sync), strictly in consumption
        # order so early phases never compete with late weights for HBM
        # bandwidth; wek/wev are issued later from the Scalar stream.
        nc.sync.dma_start(out=xts[:, 0:3, :], in_=xt[:, 0:3, :])
        nc.sync.dma_start(out=wqs[:, 0:3, :], in_=wq[:, 0:3, :])
        nc.sync.dma_start(out=xts[:, 3:6, :], in_=xt[:, 3:6, :])
        nc.sync.dma_start(out=wqs[:, 3:6, :], in_=wq[:, 3:6, :])
        nc.sync.dma_start(out=cks, in_=ckp[:, :, :])
        nc.sync.dma_start(out=cvas, in_=cva[:, :, :, :])
        nc.sync.dma_start(out=wks, in_=wk[:, :, :])
        nc.sync.dma_start(out=wvs, in_=wv[:, :, :])

        with (
            tc.tile_pool(name="scorep", bufs=2, space="PSUM") as scorep,
            tc.tile_pool(name="savp", bufs=2, space="PSUM") as savp,
            tc.tile_pool(name="auxp", bufs=2, space="PSUM") as auxp,
        ):
            def gemm_mstep(psum, lhsT_of_k, rhs_of_k, dst_copy):
                for k in range(KC):
                    nc.tensor.matmul(psum, lhsT_of_k(k), rhs_of_k(k),
                                     start=(k == 0), stop=(k == KC - 1))
                dst_copy(psum)

            # HAM warm-up: junk matmuls with no DMA deps run while the
            # input DMAs land, so the real GEMM starts closer to 2.4 GHz
            wps = auxp.tile([128, 512], F32, tag="aux", name="warm")
            for w in range(12):
                nc.tensor.matmul(wps[0:64, :NQ], ones_bc, wtile,
                                 start=True, stop=True)

            # ============ Phase A: Q projection (m-outer) ============
            for m in range(KC):
                ps = auxp.tile([128, 512], F32, tag="aux", name="qps")
                gemm_mstep(
                    ps[:, :NQ],
                    lambda k, m=m: wqs[:, k, m * 128:(m + 1) * 128],
                    lambda k: xts[:, k, :],
                    lambda p, m=m: nc.vector.tensor_copy(qts[:, m, :], p))

            # late constants (first used in phases B tail / C / D)
            onescol = pers.tile([128, 4, H, 1], F32)
            nc.vector.memset(onescol, 1.0)
            nc.vector.tensor_copy(vs[:, :, :, 64:65], onescol)
            nc.vector.tensor_copy(evs[:, :, :, 64:65], onescol)

            i65_f = pers.tile([65, 65], F32)
            nc.vector.memset(i65_f, 1.0)
            nc.gpsimd.affine_select(
                out=i65_f, in_=i65_f, compare_op=mybir.AluOpType.is_ge,
                fill=0.0, base=0, channel_multiplier=1, pattern=[[-1, 65]])
            nc.gpsimd.affine_select(
                out=i65_f, in_=i65_f, compare_op=mybir.AluOpType.is_ge,
                fill=0.0, base=0, channel_multiplier=-1, pattern=[[1, 65]])
            nc.vector.tensor_copy(i65, i65_f)

            # block-diag mask: 1.0 where key//16 == query//16 within a chunk
            bmask_f = pers.tile([128, 4, 128], F32)
            nc.vector.memset(bmask_f, 1.0)
            nc.gpsimd.affine_select(
                out=bmask_f, in_=bmask_f, compare_op=mybir.AluOpType.is_ge,
                fill=0.0, base=0, channel_multiplier=1,
                pattern=[[0, 4], [-16, 8], [0, 16]])
            nc.gpsimd.affine_select(
                out=bmask_f, in_=bmask_f, compare_op=mybir.AluOpType.is_ge,
                fill=0.0, base=15, channel_multiplier=-1,
                pattern=[[0, 4], [16, 8], [0, 16]])
            nc.vector.tensor_copy(bmask, bmask_f)

            # V[qc-chunk, hh] = sum_k XT[k][:,qc]^T @ Wv[k][:, hh*384:...]
            vsteps = [(qc, hh) for qc in range(4) for hh in range(2)]

            def v_step(qc, hh):
                cw = CH[qc]
                ps = auxp.tile([128, 512], F32, tag="aux", name="vps")
                gemm_mstep(
                    ps[:cw, :384],
                    lambda k: xts[:, k, CO[qc]:CO[qc] + cw],
                    lambda k: wvs[:, k, hh * 384:(hh + 1) * 384],
                    lambda p: nc.vector.tensor_copy(
                        vs[:cw, qc, hh * 6:(hh + 1) * 6, 0:64],
                        p.rearrange("p (h hd) -> p h hd", hd=64)))

            # ==== Phase B: cache attention head pairs + K/V GEMM filler ====
            for j in range(KC):
                he, ho = 2 * j, 2 * j + 1
                qe = qts[0:64, j, :]
                qo = qts[64:128, j, :]
                p1e = ppool.tile([128, 4, NQ], MMDT, tag="p1", name="p1e")
                p1o = ppool.tile([128, 4, NQ], MMDT, tag="p1", name="p1o")
                for half in range(2):
                    se = scorep.tile([128, 2, 512], F32, tag="score",
                                     name="s1e")
                    so = scorep.tile([128, 2, 512], F32, tag="score",
                                     name="s1o")
                    for i in range(2):
                        c = 2 * half + i
                        nc.tensor.matmul(
                            se[:, i, :NQ],
                            cks[0:64, j, c * 128:(c + 1) * 128], qe,
                            start=True, stop=True)
                        nc.tensor.matmul(
                            so[:, i, :NQ],
                            cks[64:128, j, c * 128:(c + 1) * 128], qo,
                            start=True, stop=True)
                    nc.scalar.activation(p1e[:, 2 * half:2 * half + 2, :],
                                         se[:, :, :NQ], AF.Exp, scale=SCALE)
                    nc.scalar.activation(p1o[:, 2 * half:2 * half + 2, :],
                                         so[:, :, :NQ], AF.Exp, scale=SCALE)

                # K-projection m-chunk as PE filler
                ps = auxp.tile([128, 512], F32, tag="aux", name="kps")
                gemm_mstep(
                    ps[:, :NQ],
                    lambda k, j=j: wks[:, k, j * 128:(j + 1) * 128],
                    lambda k: xts[:, k, :],
                    lambda p, j=j: nc.vector.tensor_copy(kts[:, j, :], p))

                # savU + D1 row at partition 64 (per head, K=128 full array)
                for h, p1 in ((he, p1e), (ho, p1o)):
                    savps = savp.tile([65, NQ], F32, tag="sav", name="savps")
                    for c in range(4):
                        nc.tensor.matmul(savps, cvas[:, h, c, :], p1[:, c, :],
                                         start=(c == 0), stop=(c == 3))
                    nc.vector.tensor_copy(savus[:, h, :], savps)

                # odd head's sav values land at partitions 64..127 of savn
                nc.sync.dma_start(out=savn[64:128, j, :],
                                  in_=savus[0:64, ho, :])

                # 1/D1 broadcast over the pair's partitions via K=1 matmuls
                bcp = auxp.tile([128, 512], F32, tag="aux", name="bc1")
                nc.tensor.matmul(bcp[0:64, :NQ], ones_bc[64:65, :],
                                 savus[64:65, he, :])
                nc.tensor.matmul(bcp[64:128, :NQ], ones_bc[64:65, :],
                                 savus[64:65, ho, :])
                rbc = spool.tile([128, NQ], F32, tag="rbc", name="rbc")
                nc.vector.reciprocal_approx_fast(out=rbc, in_=bcp[:, :NQ])
                rbcb = spool.tile([128, NQ], MMDT, tag="rbcb", name="rbcb")
                nc.vector.tensor_copy(rbcb, rbc)
                nc.vector.tensor_mul(savn[0:64, j, :], savus[0:64, he, :],
                                     rbcb[0:64, :])
                nc.vector.tensor_mul(savn[64:128, j, :], savn[64:128, j, :],
                                     rbcb[64:128, :])

                # late-phase weights: issued from the Scalar stream once the
                # startup-critical transfers are done (FIFO per ring)
                if j == 1:
                    nc.scalar.dma_start(out=weks, in_=wek[:, :, :])
                if j == 2:
                    nc.scalar.dma_start(out=wevs, in_=wev[:, :, :])

                # (V projection moved out of this PE-paced phase: its
                # hh=0 chunks run in phase C, hh=1 as phase-D filler)

            # ============ Phase C: EK / EV projections ============
            # ACT is idle here, so PSUM evacuation goes on nc.scalar
            for m in range(KC):
                ps = auxp.tile([128, 512], F32, tag="aux", name="ekps")
                gemm_mstep(
                    ps[:, :NQ],
                    lambda k, m=m: weks[:, k, m * 128:(m + 1) * 128],
                    lambda k: savn[:, k, :],
                    lambda p, m=m: nc.scalar.copy(ekts[:, m, :], p))

            def ev_step(qc, hh, eng):
                cw = CH[qc]
                ps = auxp.tile([128, 512], F32, tag="aux", name="evps")
                gemm_mstep(
                    ps[:cw, :384],
                    lambda k: savn[:, k, CO[qc]:CO[qc] + cw],
                    lambda k: wevs[:, k, hh * 384:(hh + 1) * 384],
                    lambda p: eng.tensor_copy(
                        evs[:cw, qc, hh * 6:(hh + 1) * 6, 0:64],
                        p.rearrange("p (h hd) -> p h hd", hd=64))
                    if eng is nc.vector else
                    nc.scalar.copy(
                        evs[:cw, qc, hh * 6:(hh + 1) * 6, 0:64],
                        p.rearrange("p (h hd) -> p h hd", hd=64)))

            # heads 0..5 (hh=0) are consumed from phase-D pair 0 on: all
            # four of their EV chunks must be issued BEFORE the first ctx
            ev_step(0, 0, nc.scalar)
            ev_step(1, 0, nc.scalar)
            ev_step(2, 0, nc.scalar)
            ev_step(3, 0, nc.scalar)
            # V hh=0 chunks: consumed by phase-D pair 0's local-context MMs
            v_step(0, 0)
            v_step(1, 0)
            v_step(2, 0)
            v_step(3, 0)

            # ============ Phase D: full softmax + context ============
            outg = None
            for j in range(KC):
                he, ho = 2 * j, 2 * j + 1
                qe = qts[0:64, j, :]
                qo = qts[64:128, j, :]
                p2e = ppool.tile([128, 4, NQ], MMDT, tag="p1", name="p2e")
                p2o = ppool.tile([128, 4, NQ], MMDT, tag="p1", name="p2o")
                for half in range(2):
                    se = scorep.tile([128, 2, 512], F32, tag="score",
                                     name="s2e")
                    so = scorep.tile([128, 2, 512], F32, tag="score",
                                     name="s2o")
                    for i in range(2):
                        c = 2 * half + i
                        cw = CH[c]
                        nc.tensor.matmul(
                            se[:cw, i, :NQ],
                            ekts[0:64, j, CO[c]:CO[c] + cw], qe,
                            start=True, stop=True)
                        nc.tensor.matmul(
                            so[:cw, i, :NQ],
                            ekts[64:128, j, CO[c]:CO[c] + cw], qo,
                            start=True, stop=True)
                    nc.scalar.activation(p2e[:, 2 * half:2 * half + 2, :],
                                         se[:, :, :NQ], AF.Exp, scale=SCALE)
                    nc.scalar.activation(p2o[:, 2 * half:2 * half + 2, :],
                                         so[:, :, :NQ], AF.Exp, scale=SCALE)

                # local block-diagonal scores, pair-interleaved (K=64)
                s3e = auxp.tile([128, 512], F32, tag="aux", name="s3e")
                s3o = auxp.tile([128, 512], F32, tag="aux", name="s3o")
                for c in range(4):
                    cw = CH[c]
                    nc.tensor.matmul(
                        s3e[:cw, c * 128:c * 128 + cw],
                        kts[0:64, j, CO[c]:CO[c] + cw],
                        qe[:, CO[c]:CO[c] + cw],
                        start=True, stop=True)
                    nc.tensor.matmul(
                        s3o[:cw, c * 128:c * 128 + cw],
                        kts[64:128, j, CO[c]:CO[c] + cw],
                        qo[:, CO[c]:CO[c] + cw],
                        start=True, stop=True)
                p3e = ppool.tile([128, 4, 128], MMDT, tag="p3", name="p3e",
                                 bufs=3)
                p3o = ppool.tile([128, 4, 128], MMDT, tag="p3", name="p3o",
                                 bufs=3)
                nc.scalar.activation(
                    p3e, s3e.rearrange("p (c n) -> p c n", n=128),
                    AF.Exp, scale=SCALE)
                nc.scalar.activation(
                    p3o, s3o.rearrange("p (c n) -> p c n", n=128),
                    AF.Exp, scale=SCALE)
                nc.vector.tensor_mul(p3e, p3e, bmask)
                nc.vector.tensor_mul(p3o, p3o, bmask)

                # EV/V hh=1 filler: all issued before the first hh=1
                # consumer (pair j=3, heads 6,7)
                if j == 0:
                    ev_step(0, 1, nc.vector)
                    ev_step(1, 1, nc.vector)
                    v_step(0, 1)
                elif j == 1:
                    ev_step(2, 1, nc.vector)
                    ev_step(3, 1, nc.vector)
                    v_step(1, 1)
                elif j == 2:
                    v_step(2, 1)
                    v_step(3, 1)

                for h, p2, p3 in ((he, p2e, p3e), (ho, p2o, p3o)):
                    ctxps = savp.tile([65, NQ], F32, tag="sav", name="ctxps")
                    # savU + D1 fold-in via identity matmul opens the group
                    nc.tensor.matmul(ctxps, i65, savus[:, h, :],
                                     start=True, stop=False)
                    for c in range(4):
                        nc.tensor.matmul(ctxps, evs[:CH[c], c, h, :],
                                         p2[:CH[c], c, :],
                                         start=False, stop=False,
                                         skip_group_check=True)
                    for c in range(4):
                        cw = CH[c]
                        nc.tensor.matmul(
                            ctxps[:, CO[c]:CO[c] + cw],
                            vs[:cw, c, h, :], p3[:cw, c, :cw],
                            start=False, stop=(c == 3), skip_group_check=True)
                    # D2 lives in ctxps row 64; stage it in SBUF for the
                    # K=1 broadcast matmul (PE reads SBUF only)
                    d2row = spool.tile([65, NQ], MMDT, tag="cts",
                                       name="d2row", bufs=3)
                    nc.vector.tensor_copy(d2row[64:65, :], ctxps[64:65, :])
                    bcp2 = auxp.tile([64, 512], F32, tag="aux", name="bc2")
                    nc.tensor.matmul(bcp2[:, :NQ], ones_bc[64:65, :],
                                     d2row[64:65, :])
                    rbc2 = spool.tile([64, NQ], F32, tag="rbc", name="rbc2")
                    nc.vector.reciprocal_approx_fast(out=rbc2,
                                                     in_=bcp2[:, :NQ])
                    if h % 2 == 0:
                        outg = outpool.tile([64, 2, NQ], F32,
                                            tag="outg", name="outg")
                    nc.vector.tensor_mul(outg[:, h % 2, :],
                                         ctxps[0:64, :], rbc2)
                    if h % 2 == 1:
                        og = h // 2
                        nc.sync.dma_start(
                            out=outt[og * 128:(og + 1) * 128, :]
                            .rearrange("(hh dd) q -> dd hh q", dd=64),
                            in_=outg)


_BUILT = None


def _get_built():
    global _BUILT
    if _BUILT is None:
        _BUILT = build_bass()
    return _BUILT


last_exec_time_ns = None


def _np_mmdt():
    return dt.np(MMDT)


def make_cva(cv_b):
    """(12, 512, 64) cache values -> ones-augmented chunk layout."""
    cva = np.ones((128, H, 4, 65), np.float32)
    cva[:, :, :, 0:64] = cv_b.reshape(H, 4, 128, HD).transpose(2, 0, 1, 3)
    return cva.astype(_np_mmdt())


def make_ckp(ck_b):
    """(12, 512, 64) cache keys -> pair-packed transposed layout."""
    ckt = ck_b.transpose(0, 2, 1)                  # (H, HD, SEQ)
    ckp = np.empty((128, KC, SEQ), np.float32)
    for j in range(KC):
        ckp[0:64, j] = ckt[2 * j]
        ckp[64:128, j] = ckt[2 * j + 1]
    return ckp.astype(_np_mmdt())


def kernel(**inputs):
    global last_exec_time_ns
    hs = np.ascontiguousarray(np.asarray(inputs['hidden_states'],
                                         dtype=np.float32))
    ck = np.asarray(inputs['cache_key'], dtype=np.float32)
    cv = np.asarray(inputs['cache_value'], dtype=np.float32)
    ws = {k: np.ascontiguousarray(np.asarray(inputs[k], dtype=np.float32))
          for k in ('Wq', 'Wk', 'Wv', 'Wek', 'Wev')}

    for name in ('attention_mask', 'slot_unified_mask', 'bq', 'bk', 'bv',
                 'bek', 'bev'):
        if name in inputs and np.abs(np.asarray(inputs[name])).max() != 0:
            print(f"WARNING: kernel assumes {name} == 0 but it is not; "
                  f"results will be wrong")

    nc = _get_built()

    def sb_layout(a, inner):
        # (D, inner) -> contiguous (128, KC, inner) SBUF image
        return np.ascontiguousarray(
            a.reshape(KC, 128, inner).transpose(1, 0, 2)).astype(_np_mmdt())

    wsc = {k: sb_layout(w, D) for k, w in ws.items()}

    hs_r = hs.reshape(S, B, L, D)
    in_maps = []
    for b in range(N_CORES):
        in_maps.append({
            'xt': sb_layout(hs_r[:, b].reshape(NQ, D).T, NQ),
            'ckp': make_ckp(ck[b]),
            'cva': make_cva(cv[b]),
            'wq': wsc['Wq'], 'wk': wsc['Wk'], 'wv': wsc['Wv'],
            'wek': wsc['Wek'], 'wev': wsc['Wev'],
        })

    res = run_bass_kernel_spmd(
        nc, in_maps, core_ids=list(range(N_CORES)),
        trace=bool(os.environ.get("BASS_TRACE")),
        tmpdir=os.environ.get("BASS_TMPDIR"))
    last_exec_time_ns = res.exec_time_ns

    out = np.zeros((S, B, L, D), np.float32)
    for b in range(N_CORES):
        out[:, b] = res.results[b]['outt'].astype(np.float32).T.reshape(
            S, L, D)
    return out.reshape(S * B, L, D)
